# revision 2
# baseline (speedup 1.0000x reference)
"""DBRX-experts MoE kernel for 8 Trainium2 NeuronCores (expert-parallel).

Strategy
--------
E=8 experts map 1:1 onto the 8 cores. The host gathers each expert's routed
tokens (top-k dispatch done in numpy — the "all-to-all" of the sharding hint
collapses to a host-side gather because kernel() already owns the full
inputs), pads them to a common count, and pre-lays-out the expert's weights
so the device kernel is a pure dense transposed MLP:

    G^T = Wg^T-tiles @ X^T      (contract H, out [F, T])
    U^T = Wu^T-tiles @ X^T
    Hmid^T = silu(G^T) * U^T
    Y^T = Wd^T-tiles @ Hmid^T   (contract F, out [H, T])

All tensors are bf16 (PSUM accumulation in fp32; rel err ~4e-3 vs the fp32
reference, comfortably under the 2e-2 gate). bf16 halves the weight-stream
DMA (25MB/core/dispatch) vs fp32r, which removes the prologue DMA wall, and
runs the PE at the same 1 row/cycle. Matmuls keep weights stationary
([128,128] tiles) with tokens moving ([128, n_pad]); n_pad is the max routed
count rounded up to a multiple of 4 (bf16 has no >=256 free-dim requirement,
so no padding to 512 — ~2.5% fewer PE rows).

Schedule notes (from timeline-sim tuning):
- A short junk-matmul warmup chain runs while the first DMAs land, keeping
  the PE p-state ramped so real matmuls start at full clock.
- X^T goes on the scalar(Act) HWDGE queue in 4 slab-group DMAs; the first 4
  weight tiles are quartered on the sync(SP) HWDGE queue; remaining weight
  tiles stream on the gpsimd(Pool) SWDGE queue, throttled by a 3-buffer
  weight pool so they cannot hog the DMA bus ahead of X^T during the
  prologue.
- silu is computed directly on the Activation engine (one table load, no
  swaps; `copy` lives in the same table for the output-tail copies).
- The last output slab's PSUM->SBUF copy is split across DVE and Act to
  shorten the kernel tail.

The per-token combine weights and the scatter-add back into the [T, H]
output (the "all-reduce") are applied on the host. No device collectives:
cores are fully independent.
"""

import os

# The axon jax platform must stay visible even if the caller pinned cpu for
# its own reference computation (bass2jax needs jax.devices() -> axon).
if os.environ.get("JAX_PLATFORMS") == "cpu":
    os.environ["JAX_PLATFORMS"] = ""

import numpy as np

import concourse.bass as bass
import concourse.mybir as mybir
import concourse.tile as tile
from concourse.bass_utils import run_bass_kernel_spmd

E, H, F, P = 8, 2048, 2048, 128
HO, FO = H // P, F // P  # 16, 16

F32 = mybir.dt.float32
F32R = mybir.dt.float32r
BF16 = mybir.dt.bfloat16

ACT_SILU = mybir.ActivationFunctionType.Silu
ACT_COPY = mybir.ActivationFunctionType.Copy

_prog_cache: dict = {}


def _pad_count(maxc: int) -> int:
    """Tokens padded to a multiple of 4 (DMA-friendly 8B runs in bf16)."""
    return max(8, -(-maxc // 4) * 4)


def _chunks_for(n_pad: int):
    """Split [0, n_pad) into equal chunks of <=500 (PSUM bank = 512 fp32;
    500 keeps a bank per chunk with 4-token alignment)."""
    n_ch = -(-n_pad // 500)
    cn = -(-n_pad // n_ch)
    cn = -(-cn // 4) * 4
    out = []
    c0 = 0
    while c0 < n_pad:
        out.append((c0, min(cn, n_pad - c0)))
        c0 += cn
    return out


def _legalize_sync_waits(nc):
    """Split sync waits exceeding the per-instruction ISA budget into NOPs.

    This walrus build rejects instructions with too many embedded sync-wait
    commands ("Too many sync wait commands", CoreV3GenImpl setupSyncWait):
    Matmult tolerates 1, most opcodes 2, and Tile's scheduler freely emits
    more (e.g. the kernel-tail Drain). Moving the excess waits onto NoOp
    instructions placed immediately before the offender on the same engine
    queue is semantically identical: the engine blocks on the NOP first,
    then issues the original instruction.
    """
    ctr = 0
    for fn in nc.m.functions:
        for blk in fn.blocks:
            insts = blk.instructions
            out = []
            changed = False
            for inst in insts:
                si = inst.sync_info
                waits = list(si.on_wait) if si is not None and si.on_wait else []
                limit = 1
                if len(waits) > limit:
                    extra, keep = waits[:-limit], waits[-limit:]
                    for w in extra:
                        nop = mybir.InstNoOp(
                            name=f"ant_sync_split_{ctr}", ins=[], outs=[]
                        )
                        ctr += 1
                        nop.engine = inst.engine
                        nop.sync_info = mybir.SyncInfo(on_wait=[w], on_update=[])
                        out.append(nop)
                    si.on_wait = keep
                    changed = True
                out.append(inst)
            if changed:
                blk.instructions = out


def _build_program(
    n_pad: int,
    use_bf16: bool = True,
    legalize: bool = True,
    reps: int = 1,
    warm_n: int = 8,
    warm_rows: int = 256,
    wpool_bufs: int = 3,
    tpool_bufs: int = 3,
    xt_dmas: int = 4,
    wq_hwdge: int = 4,
):
    dt_in = BF16 if use_bf16 else F32R
    dt_out = BF16 if use_bf16 else F32

    nc = bass.Bass("TRN2")

    xt_d = nc.dram_tensor("xt", [P, HO, n_pad], dt_in, kind="ExternalInput")
    # gate and up interleaved on the second-to-last axis: one DMA per fo
    wgu_d = nc.dram_tensor("wgu", [FO, P, HO, 2, P], dt_in, kind="ExternalInput")
    wd_d = nc.dram_tensor("wd", [HO, P, FO, P], dt_in, kind="ExternalInput")
    yt_d = nc.dram_tensor("yt", [P, HO, n_pad], dt_out, kind="ExternalOutput")

    with tile.TileContext(nc) as tc:
        with (
            tc.tile_pool(name="xpool", bufs=2) as xpool,
            tc.tile_pool(name="wpool", bufs=wpool_bufs) as wpool,
            tc.tile_pool(name="hpool", bufs=1) as hpool,
            tc.tile_pool(name="tpool", bufs=tpool_bufs) as tpool,
            tc.tile_pool(name="pp", bufs=2, space="PSUM") as pp,
        ):
            if warm_n:
                # PE warmup on zeroed junk while the first DMAs land.
                wz = tpool.tile([P, warm_rows + P], dt_in, tag="warm")
                nc.vector.memset(wz[:], 0)
                for _i in range(warm_n):
                    pw = pp.tile([P, warm_rows], F32, tag="pwarm")
                    nc.tensor.matmul(
                        pw, wz[:, :P], wz[:, P:], start=True, stop=True
                    )

            for _rep in range(reps):
                _emit_body(
                    nc, n_pad, dt_in, dt_out,
                    xt_d, wgu_d, wd_d, yt_d,
                    xpool, wpool, hpool, tpool, pp,
                    xt_dmas=xt_dmas,
                    wq_hwdge=wq_hwdge,
                )

    if legalize:
        _legalize_sync_waits(nc)
    return nc


def _emit_body(
    nc, n_pad, dt_in, dt_out,
    xt_d, wgu_d, wd_d, yt_d,
    xpool, wpool, hpool, tpool, pp,
    xt_dmas=4,
    wq_hwdge=4,
):
    chunks = _chunks_for(n_pad)

    # X^T resident in SBUF [hi, ho, t]; slab-group DMAs on the scalar
    # (Act) queue so next-body prefetch never queues behind output DMAs.
    xt = xpool.tile([P, HO, n_pad], dt_in, tag="xt")
    assert HO % xt_dmas == 0
    grp = HO // xt_dmas

    # First weight tiles quartered on the sync (SP) HWDGE queue: the PE's
    # first matmul waits only for a quarter, and the big SWDGE weight tiles
    # can't monopolize the bus ahead of X^T.
    wgu_tiles = {}
    for fo in range(wq_hwdge):
        wgu_tiles[fo] = wpool.tile(
            [P, HO, 2, P], dt_in, tag="wgu", name=f"wgu_pre{fo}"
        )

    for i in range(xt_dmas):
        s = i * grp
        nc.scalar.dma_start(xt[:, s : s + grp], xt_d[:, s : s + grp])
    for fo in range(wq_hwdge):
        for q in range(4):
            nc.sync.dma_start(
                wgu_tiles[fo][:, 4 * q : 4 * q + 4],
                wgu_d[fo][:, 4 * q : 4 * q + 4],
            )

    hmid = hpool.tile([P, FO, n_pad], dt_in, tag="hmid")

    # Phase 1: G^T/U^T per 128-row slab of F, fused silu*up -> hmid
    for fo in range(FO):
        if fo in wgu_tiles:
            wgu_t = wgu_tiles[fo]
        else:
            wgu_t = wpool.tile([P, HO, 2, P], dt_in, tag="wgu")
            nc.gpsimd.dma_start(wgu_t[:], wgu_d[fo])
        for c0, cn in chunks:
            pg = pp.tile([P, cn], F32, tag="pg")
            pu = pp.tile([P, cn], F32, tag="pu")
            for ho in range(HO):
                nc.tensor.matmul(
                    pg, wgu_t[:, ho, 0], xt[:, ho, c0 : c0 + cn],
                    start=ho == 0, stop=ho == HO - 1,
                )
            for ho in range(HO):
                nc.tensor.matmul(
                    pu, wgu_t[:, ho, 1], xt[:, ho, c0 : c0 + cn],
                    start=ho == 0, stop=ho == HO - 1,
                )
            sl = tpool.tile([P, cn], F32, tag="sl")
            nc.scalar.activation(sl, pg, ACT_SILU)
            nc.vector.tensor_mul(
                out=hmid[:, fo, c0 : c0 + cn], in0=sl, in1=pu
            )

    # Phase 2: Y^T per 128-row slab of H
    for ho in range(HO):
        wd_t = wpool.tile([P, FO, P], dt_in, tag="wd")
        nc.gpsimd.dma_start(wd_t[:], wd_d[ho])
        last_ho = ho == HO - 1
        for ci, (c0, cn) in enumerate(chunks):
            last = last_ho and ci == len(chunks) - 1
            py = pp.tile([P, cn], F32, tag="py")
            for fo in range(FO):
                nc.tensor.matmul(
                    py, wd_t[:, fo], hmid[:, fo, c0 : c0 + cn],
                    start=fo == 0, stop=fo == FO - 1,
                )
            yo = tpool.tile([P, cn], dt_out, tag="yo")
            if last:
                # split the tail copy across DVE and Act
                h1 = cn // 2
                nc.vector.tensor_copy(out=yo[:, :h1], in_=py[:, :h1])
                nc.scalar.activation(yo[:, h1:], py[:, h1:], ACT_COPY)
            else:
                nc.vector.tensor_copy(out=yo[:], in_=py)
            nc.sync.dma_start(yt_d[:, ho, c0 : c0 + cn], yo)


def _get_program(n_pad: int, use_bf16: bool = True, **kw):
    key = (n_pad, use_bf16, tuple(sorted(kw.items())))
    if key not in _prog_cache:
        _prog_cache[key] = _build_program(n_pad, use_bf16, **kw)
    return _prog_cache[key]


def _route(top_experts: np.ndarray, top_weights: np.ndarray):
    """Per-expert token indices and combined weights (duplicates merged)."""
    te = np.asarray(top_experts).astype(np.int64)
    tw = np.asarray(top_weights, dtype=np.float32)
    idx_list, w_list = [], []
    for e in range(E):
        m = te == e
        sel = m.any(axis=1)
        idx = np.nonzero(sel)[0]
        w = (tw * m).sum(axis=1)[idx].astype(np.float32)
        idx_list.append(idx)
        w_list.append(w)
    return idx_list, w_list


def _np_dt(use_bf16: bool):
    if use_bf16:
        import ml_dtypes

        return np.dtype(ml_dtypes.bfloat16)
    return np.dtype(np.float32)


def _make_in_map(x, w_gate_e, w_up_e, w_down_e, idx, n_pad, use_bf16):
    npdt = _np_dt(use_bf16)
    n_e = len(idx)
    xt = np.zeros((H, n_pad), np.float32)
    if n_e:
        xt[:, :n_e] = x[idx].T
    # [h, t] -> [hi, ho, t]
    xt_dev = np.ascontiguousarray(
        xt.reshape(HO, P, n_pad).transpose(1, 0, 2)
    ).astype(npdt)
    # Wg[f, h] -> [fo, hi, ho, fi]  (lhsT tiles [hi, fi] for each (fo, ho));
    # gate and up stacked on a new axis -> [fo, hi, ho, 2, fi]
    wg_dev = (
        np.asarray(w_gate_e, np.float32).reshape(FO, P, HO, P).transpose(0, 3, 2, 1)
    )
    wu_dev = (
        np.asarray(w_up_e, np.float32).reshape(FO, P, HO, P).transpose(0, 3, 2, 1)
    )
    wgu_dev = np.ascontiguousarray(
        np.stack([wg_dev, wu_dev], axis=3)
    ).astype(npdt)
    # Wd[h, f] -> [ho, fi, fo, hi]  (lhsT tiles [fi, hi] for each (ho, fo))
    wd_dev = np.ascontiguousarray(
        np.asarray(w_down_e, np.float32)
        .reshape(HO, P, FO, P)
        .transpose(0, 3, 2, 1)
    ).astype(npdt)
    return {"xt": xt_dev, "wgu": wgu_dev, "wd": wd_dev}


def run(
    hidden_states,
    top_weights,
    w_gate,
    w_up,
    w_down,
    top_experts,
    use_bf16: bool = True,
    **spmd_kwargs,
):
    """Full MoE forward. Returns (output, BassKernelResults)."""
    x = np.asarray(hidden_states, dtype=np.float32).reshape(-1, H)
    T = x.shape[0]

    idx_list, w_list = _route(top_experts, top_weights)
    maxc = max(len(i) for i in idx_list)
    n_pad = _pad_count(maxc)

    nc = _get_program(n_pad, use_bf16)

    in_maps = [
        _make_in_map(
            x, w_gate[e], w_up[e], w_down[e], idx_list[e], n_pad, use_bf16
        )
        for e in range(E)
    ]

    # Transient NRT exec failures (NRT_EXEC_UNIT_UNRECOVERABLE) have been
    # observed on the first 8-core execution of a fresh NEFF; retries clear
    # them.
    last_exc = None
    for _attempt in range(3):
        try:
            res = run_bass_kernel_spmd(
                nc, in_maps, core_ids=list(range(E)), **spmd_kwargs
            )
            break
        except Exception as exc:
            last_exc = exc
            import time as _time

            _time.sleep(5)
    else:
        raise last_exc

    out = np.zeros((T, H), np.float32)
    for e in range(E):
        idx = idx_list[e]
        if len(idx) == 0:
            continue
        yt = np.asarray(res.results[e]["yt"], dtype=np.float32)  # [hi, ho, t]
        y = yt.transpose(1, 0, 2).reshape(H, n_pad)[:, : len(idx)]  # [H, n_e]
        out[idx] += w_list[e][:, None] * y.T
    return out.reshape(np.asarray(hidden_states).shape).astype(np.float32), res


def kernel(hidden_states, top_weights, w_gate, w_up, w_down, top_experts):
    out, _ = run(hidden_states, top_weights, w_gate, w_up, w_down, top_experts)
    return out


# revision 16
# speedup vs baseline: 1.8326x; 1.8326x over previous
"""DBRX-experts MoE kernel for 8 Trainium2 NeuronCores (expert-parallel).

Strategy
--------
E=8 experts map 1:1 onto the 8 cores. The host gathers each expert's routed
tokens (top-k dispatch done in numpy — the "all-to-all" of the sharding hint
collapses to a host-side gather because kernel() already owns the full
inputs), pads them to a common count, and pre-lays-out the expert's weights
so the device kernel is a pure dense transposed MLP:

    G^T = Wg^T-tiles @ X^T      (contract H, out [F, T])
    U^T = Wu^T-tiles @ X^T
    Hmid^T = silu(G^T) * U^T
    Y^T = Wd^T-tiles @ Hmid^T   (contract F, out [H, T])

All tensors are bf16 (PSUM accumulation in fp32; rel err ~4e-3 vs the fp32
reference, comfortably under the 2e-2 gate). bf16 halves the weight-stream
DMA (25MB/core/dispatch) vs fp32r, which removes the prologue DMA wall, and
runs the PE at the same 1 row/cycle. Matmuls keep weights stationary
([128,128] tiles) with tokens moving ([128, n_pad]); n_pad is the max routed
count rounded up to a multiple of 4 (bf16 has no >=256 free-dim requirement,
so no padding to 512 — ~2.5% fewer PE rows).

Schedule notes (from timeline-sim tuning):
- A short junk-matmul warmup chain runs while the first DMAs land, keeping
  the PE p-state ramped so real matmuls start at full clock.
- X^T goes on the scalar(Act) HWDGE queue in 4 slab-group DMAs; the first 4
  weight tiles are quartered on the sync(SP) HWDGE queue; remaining weight
  tiles stream on the gpsimd(Pool) SWDGE queue, throttled by a 3-buffer
  weight pool so they cannot hog the DMA bus ahead of X^T during the
  prologue.
- silu is computed directly on the Activation engine (one table load, no
  swaps; `copy` lives in the same table for the output-tail copies).
- The last output slab's PSUM->SBUF copy is split across DVE and Act to
  shorten the kernel tail.

The per-token combine weights and the scatter-add back into the [T, H]
output (the "all-reduce") are applied on the host. No device collectives:
cores are fully independent.
"""

import os

# The axon jax platform must stay visible even if the caller pinned cpu for
# its own reference computation (bass2jax needs jax.devices() -> axon).
if os.environ.get("JAX_PLATFORMS") == "cpu":
    os.environ["JAX_PLATFORMS"] = ""

import numpy as np

import concourse.bass as bass
import concourse.mybir as mybir
import concourse.tile as tile
from concourse.bass_utils import run_bass_kernel_spmd

E, H, F, P = 8, 2048, 2048, 128
HO, FO = H // P, F // P  # 16, 16

F32 = mybir.dt.float32
F32R = mybir.dt.float32r
BF16 = mybir.dt.bfloat16

ACT_SILU = mybir.ActivationFunctionType.Silu
ACT_COPY = mybir.ActivationFunctionType.Copy

_prog_cache: dict = {}


def _pad_count(maxc: int) -> int:
    """Tokens padded to a multiple of 4 (DMA-friendly 8B runs in bf16)."""
    return max(8, -(-maxc // 4) * 4)


def _chunks_for(n_pad: int):
    """Split [0, n_pad) into equal chunks of <=500 (PSUM bank = 512 fp32;
    500 keeps a bank per chunk with 4-token alignment)."""
    n_ch = -(-n_pad // 500)
    cn = -(-n_pad // n_ch)
    cn = -(-cn // 4) * 4
    out = []
    c0 = 0
    while c0 < n_pad:
        out.append((c0, min(cn, n_pad - c0)))
        c0 += cn
    return out


def _legalize_sync_waits(nc):
    """Split sync waits exceeding the per-instruction ISA budget into NOPs.

    This walrus build rejects instructions with too many embedded sync-wait
    commands ("Too many sync wait commands", CoreV3GenImpl setupSyncWait):
    Matmult tolerates 1, most opcodes 2, and Tile's scheduler freely emits
    more (e.g. the kernel-tail Drain). Moving the excess waits onto NoOp
    instructions placed immediately before the offender on the same engine
    queue is semantically identical: the engine blocks on the NOP first,
    then issues the original instruction.
    """
    ctr = 0
    for fn in nc.m.functions:
        for blk in fn.blocks:
            insts = blk.instructions
            out = []
            changed = False
            for inst in insts:
                si = inst.sync_info
                waits = list(si.on_wait) if si is not None and si.on_wait else []
                limit = 1
                if len(waits) > limit:
                    extra, keep = waits[:-limit], waits[-limit:]
                    for w in extra:
                        nop = mybir.InstNoOp(
                            name=f"ant_sync_split_{ctr}", ins=[], outs=[]
                        )
                        ctr += 1
                        nop.engine = inst.engine
                        nop.sync_info = mybir.SyncInfo(on_wait=[w], on_update=[])
                        out.append(nop)
                    si.on_wait = keep
                    changed = True
                out.append(inst)
            if changed:
                blk.instructions = out


def _build_program(
    n_pad: int,
    use_bf16: bool = True,
    legalize: bool = True,
    reps: int = 1,
    warm_n: int = 8,
    warm_rows: int = 256,
    wpool_bufs: int = 3,
    tpool_bufs: int = 3,
    xt_dmas: int = 4,
    wq_hwdge: int = 4,
    diag: str | None = None,
    loop_n: int = 0,
    wq_spread: int = 1,
):
    dt_in = BF16 if use_bf16 else F32R
    dt_out = BF16 if use_bf16 else F32

    nc = bass.Bass("TRN2")

    xt_d = nc.dram_tensor("xt", [P, HO, n_pad], dt_in, kind="ExternalInput")
    # gate and up interleaved on the second-to-last axis: one DMA per fo
    wgu_d = nc.dram_tensor("wgu", [FO, P, HO, 2, P], dt_in, kind="ExternalInput")
    wd_d = nc.dram_tensor("wd", [HO, P, FO, P], dt_in, kind="ExternalInput")
    yt_d = nc.dram_tensor("yt", [P, HO, n_pad], dt_out, kind="ExternalOutput")

    with tile.TileContext(nc) as tc:
        with (
            tc.tile_pool(name="xpool", bufs=2) as xpool,
            tc.tile_pool(name="wpool", bufs=wpool_bufs) as wpool,
            tc.tile_pool(name="hpool", bufs=1) as hpool,
            tc.tile_pool(name="tpool", bufs=tpool_bufs) as tpool,
            tc.tile_pool(name="pp", bufs=2, space="PSUM") as pp,
        ):
            if warm_n:
                # PE warmup on zeroed junk while the first DMAs land
                # (always bf16: DVE memset rejects float32r).
                wz = tpool.tile([P, warm_rows + P], BF16, tag="warm")
                nc.vector.memset(wz[:], 0)
                for _i in range(warm_n):
                    pw = pp.tile([P, warm_rows], F32, tag="pwarm")
                    nc.tensor.matmul(
                        pw, wz[:, :P], wz[:, P:], start=True, stop=True
                    )

            body_kw = dict(
                xt_dmas=xt_dmas, wq_hwdge=wq_hwdge, diag=diag,
                wq_spread=wq_spread,
            )
            if loop_n:
                # Hardware loop: body emitted once, executed loop_n times
                # on device (for RPC-noise-immune timing; the result is
                # identical every iteration).
                with tc.For_i(0, loop_n):
                    _emit_body(
                        nc, n_pad, dt_in, dt_out,
                        xt_d, wgu_d, wd_d, yt_d,
                        xpool, wpool, hpool, tpool, pp, **body_kw,
                    )
            else:
                for _rep in range(reps):
                    _emit_body(
                        nc, n_pad, dt_in, dt_out,
                        xt_d, wgu_d, wd_d, yt_d,
                        xpool, wpool, hpool, tpool, pp, **body_kw,
                    )

    if legalize:
        _legalize_sync_waits(nc)
    return nc


def _emit_body(
    nc, n_pad, dt_in, dt_out,
    xt_d, wgu_d, wd_d, yt_d,
    xpool, wpool, hpool, tpool, pp,
    xt_dmas=4,
    wq_hwdge=4,
    diag=None,
    wq_spread=1,
):
    # diag="dmaonly": all DMAs, no compute (DMA/queue roofline probe)
    # diag="reusew":  full compute, weight tiles loaded once (PE roofline
    #                 probe; wrong numerics, timing only)
    chunks = _chunks_for(n_pad)

    # Weight-stream queues, round-robined when wq_spread > 1 (per-queue
    # DMA-engine assignment caps single-queue bandwidth on real HW).
    # Only gpsimd (SWDGE) + SP/Activation (HWDGE) can initiate DMAs.
    w_engines = [nc.gpsimd, nc.scalar, nc.sync][:wq_spread]

    def w_dma(i, dst, src):
        w_engines[i % len(w_engines)].dma_start(dst, src)

    # X^T resident in SBUF [hi, ho, t]; slab-group DMAs on the scalar
    # (Act) queue so next-body prefetch never queues behind output DMAs.
    xt = xpool.tile([P, HO, n_pad], dt_in, tag="xt")
    assert HO % xt_dmas == 0
    grp = HO // xt_dmas

    # First weight tiles quartered on the sync (SP) HWDGE queue: the PE's
    # first matmul waits only for a quarter, and the big SWDGE weight tiles
    # can't monopolize the bus ahead of X^T.
    wgu_tiles = {}
    for fo in range(wq_hwdge):
        wgu_tiles[fo] = wpool.tile(
            [P, HO, 2, P], dt_in, tag="wgu", name=f"wgu_pre{fo}"
        )

    for i in range(xt_dmas):
        s = i * grp
        nc.scalar.dma_start(xt[:, s : s + grp], xt_d[:, s : s + grp])
    for fo in range(wq_hwdge):
        for q in range(4):
            nc.sync.dma_start(
                wgu_tiles[fo][:, 4 * q : 4 * q + 4],
                wgu_d[fo][:, 4 * q : 4 * q + 4],
            )

    hmid = hpool.tile([P, FO, n_pad], dt_in, tag="hmid")

    if diag == "dmaonly":
        yz = tpool.tile([P, n_pad], dt_out, tag="yz")
        nc.vector.memset(yz[:], 0)
        for fo in range(wq_hwdge, FO):
            wgu_t = wpool.tile([P, HO, 2, P], dt_in, tag="wgu")
            w_dma(fo, wgu_t[:], wgu_d[fo])
        for ho in range(HO):
            wd_t = wpool.tile([P, FO, P], dt_in, tag="wd")
            w_dma(FO + ho, wd_t[:], wd_d[ho])
        for ho in range(HO):
            for c0, cn in chunks:
                nc.sync.dma_start(yt_d[:, ho, c0 : c0 + cn], yz[:, c0 : c0 + cn])
        return

    # Phase 1: G^T/U^T per 128-row slab of F, fused silu*up -> hmid
    for fo in range(FO):
        if fo in wgu_tiles:
            wgu_t = wgu_tiles[fo]
        elif diag == "reusew":
            wgu_t = wgu_tiles[0]
        else:
            wgu_t = wpool.tile([P, HO, 2, P], dt_in, tag="wgu")
            w_dma(fo, wgu_t[:], wgu_d[fo])
        for c0, cn in chunks:
            pg = pp.tile([P, cn], F32, tag="pg")
            pu = pp.tile([P, cn], F32, tag="pu")
            for ho in range(HO):
                nc.tensor.matmul(
                    pg, wgu_t[:, ho, 0], xt[:, ho, c0 : c0 + cn],
                    start=ho == 0, stop=ho == HO - 1,
                )
            for ho in range(HO):
                nc.tensor.matmul(
                    pu, wgu_t[:, ho, 1], xt[:, ho, c0 : c0 + cn],
                    start=ho == 0, stop=ho == HO - 1,
                )
            sl = tpool.tile([P, cn], F32, tag="sl")
            nc.scalar.activation(sl, pg, ACT_SILU)
            nc.vector.tensor_mul(
                out=hmid[:, fo, c0 : c0 + cn], in0=sl, in1=pu
            )

    # Phase 2: Y^T per 128-row slab of H
    wd_first = None
    for ho in range(HO):
        if diag == "reusew" and wd_first is not None:
            wd_t = wd_first
        else:
            wd_t = wpool.tile([P, FO, P], dt_in, tag="wd")
            w_dma(FO + ho, wd_t[:], wd_d[ho])
            wd_first = wd_t
        last_ho = ho == HO - 1
        for ci, (c0, cn) in enumerate(chunks):
            last = last_ho and ci == len(chunks) - 1
            py = pp.tile([P, cn], F32, tag="py")
            for fo in range(FO):
                nc.tensor.matmul(
                    py, wd_t[:, fo], hmid[:, fo, c0 : c0 + cn],
                    start=fo == 0, stop=fo == FO - 1,
                )
            yo = tpool.tile([P, cn], dt_out, tag="yo")
            if last:
                # split the tail copy across DVE and Act
                h1 = cn // 2
                nc.vector.tensor_copy(out=yo[:, :h1], in_=py[:, :h1])
                nc.scalar.activation(yo[:, h1:], py[:, h1:], ACT_COPY)
            else:
                nc.vector.tensor_copy(out=yo[:], in_=py)
            nc.sync.dma_start(yt_d[:, ho, c0 : c0 + cn], yo)


def _get_program(n_pad: int, use_bf16: bool = True, **kw):
    key = (n_pad, use_bf16, tuple(sorted(kw.items())))
    if key not in _prog_cache:
        _prog_cache[key] = _build_program(n_pad, use_bf16, **kw)
    return _prog_cache[key]


def _route(top_experts: np.ndarray, top_weights: np.ndarray):
    """Per-expert token indices and combined weights (duplicates merged)."""
    te = np.asarray(top_experts).astype(np.int64)
    tw = np.asarray(top_weights, dtype=np.float32)
    idx_list, w_list = [], []
    for e in range(E):
        m = te == e
        sel = m.any(axis=1)
        idx = np.nonzero(sel)[0]
        w = (tw * m).sum(axis=1)[idx].astype(np.float32)
        idx_list.append(idx)
        w_list.append(w)
    return idx_list, w_list


def _np_dt(use_bf16: bool):
    if use_bf16:
        import ml_dtypes

        return np.dtype(ml_dtypes.bfloat16)
    return np.dtype(np.float32)


def _make_in_map(x, w_gate_e, w_up_e, w_down_e, idx, n_pad, use_bf16):
    npdt = _np_dt(use_bf16)
    n_e = len(idx)
    xt = np.zeros((H, n_pad), np.float32)
    if n_e:
        xt[:, :n_e] = x[idx].T
    # [h, t] -> [hi, ho, t]
    xt_dev = np.ascontiguousarray(
        xt.reshape(HO, P, n_pad).transpose(1, 0, 2)
    ).astype(npdt)
    # Wg[f, h] -> [fo, hi, ho, fi]  (lhsT tiles [hi, fi] for each (fo, ho));
    # gate and up stacked on a new axis -> [fo, hi, ho, 2, fi]
    wg_dev = (
        np.asarray(w_gate_e, np.float32).reshape(FO, P, HO, P).transpose(0, 3, 2, 1)
    )
    wu_dev = (
        np.asarray(w_up_e, np.float32).reshape(FO, P, HO, P).transpose(0, 3, 2, 1)
    )
    wgu_dev = np.ascontiguousarray(
        np.stack([wg_dev, wu_dev], axis=3)
    ).astype(npdt)
    # Wd[h, f] -> [ho, fi, fo, hi]  (lhsT tiles [fi, hi] for each (ho, fo))
    wd_dev = np.ascontiguousarray(
        np.asarray(w_down_e, np.float32)
        .reshape(HO, P, FO, P)
        .transpose(0, 3, 2, 1)
    ).astype(npdt)
    return {"xt": xt_dev, "wgu": wgu_dev, "wd": wd_dev}


def run(
    hidden_states,
    top_weights,
    w_gate,
    w_up,
    w_down,
    top_experts,
    use_bf16: bool = True,
    **spmd_kwargs,
):
    """Full MoE forward. Returns (output, BassKernelResults)."""
    x = np.asarray(hidden_states, dtype=np.float32).reshape(-1, H)
    T = x.shape[0]

    idx_list, w_list = _route(top_experts, top_weights)
    maxc = max(len(i) for i in idx_list)
    n_pad = _pad_count(maxc)

    nc = _get_program(n_pad, use_bf16)

    in_maps = [
        _make_in_map(
            x, w_gate[e], w_up[e], w_down[e], idx_list[e], n_pad, use_bf16
        )
        for e in range(E)
    ]

    # Transient NRT exec failures (NRT_EXEC_UNIT_UNRECOVERABLE) have been
    # observed on the first 8-core execution of a fresh NEFF; retries clear
    # them.
    last_exc = None
    for _attempt in range(3):
        try:
            res = run_bass_kernel_spmd(
                nc, in_maps, core_ids=list(range(E)), **spmd_kwargs
            )
            break
        except Exception as exc:
            last_exc = exc
            import time as _time

            _time.sleep(5)
    else:
        raise last_exc

    out = np.zeros((T, H), np.float32)
    for e in range(E):
        idx = idx_list[e]
        if len(idx) == 0:
            continue
        yt = np.asarray(res.results[e]["yt"], dtype=np.float32)  # [hi, ho, t]
        y = yt.transpose(1, 0, 2).reshape(H, n_pad)[:, : len(idx)]  # [H, n_e]
        out[idx] += w_list[e][:, None] * y.T
    return out.reshape(np.asarray(hidden_states).shape).astype(np.float32), res


def kernel(hidden_states, top_weights, w_gate, w_up, w_down, top_experts):
    out, _ = run(hidden_states, top_weights, w_gate, w_up, w_down, top_experts)
    return out
